# revision 10
# baseline (speedup 1.0000x reference)
"""Trainium2 Bass kernel: batched inverse of homogeneous affine transforms.

Problem: trf (B, 3, 4) fp32 "shift" affines. Padded M = [[I3 + dA, t], [0, 1]].
Output = top 3 rows of M^-1 = [A^-1 | -A^-1 t] where A = I3 + dA.

Closed form via the column-cross-product adjugate:
    Z[3r+j]  = P[3r+j] - Q[3r+j]   (cross(a_{r+1}, a_{r+2}) components)
    det      = a1 . Z[0:3] ; O = Z * (1/det) ; w_r = sum_j O[r][j] * (-t_j)

Layout: PLANAR per partition. Host pre-permutes each core's (BL, 12) slab to
(nch, 128, 12, C): partition p holds 12 contiguous planes of C consecutive
matrices. Every engine op then runs on dense step-1 inner runs (measured ~2x
faster on DVE than the stride-12 interleaved layout) while each DMA still
moves one contiguous 24KB run per partition.

Plane permutation POS (slot -> matrix position) was chosen by combinatorial
search so the 9 Q-products batch as 3 ops, P row 0 batches, and tm batches
(arithmetic-progression plane strides). Work is split DVE / GPSIMD / ACT to
balance engine busy time; all pools are double-buffered so chunks pipeline.
"""

import numpy as np

B = 4_194_304
NCORES = 8
BL = B // NCORES  # 524288 matrices per core
P = 128
C = 512           # matrices per partition per chunk
NCH = BL // (P * C)  # 8 chunks

# slot -> input position (position = 4*r + col, row-major (3,4))
POS = [5, 0, 4, 8, 9, 10, 2, 6, 1, 3, 7, 11]
# output plane k -> output position: planes 0..8 = O[r][j] at 4r+j, 9..11 = w_r
OPOS = [0, 1, 2, 4, 5, 6, 8, 9, 10, 3, 7, 11]

# P products (out plane 3r+j in po block), as (out, in0_slot, in1_slot):
# row 0 batched: out {0,1,2}, in0 [0,4,8] (step 4), in1 [5,6,7] (step 1)
# rows 1,2 as pairs (2-term progressions are always affine):
# (out_base, out_step, in0_base, in0_step, in1_base, in1_step)
P_PAIRS = [
    (3, 1, 7, -2, 3, -2),   # (3,7,3), (4,5,1)
    (5, 1, 6, -4, 2, 2),    # (5,6,2), (6,2,4)
    (7, 1, 3, -2, 8, -8),   # (7,3,8), (8,1,0)
]
# Q products batched by j: (out_base, out_step, in0_base, in0_step, in1_base, in1_step)
Q_BATCH = [
    (0, 3, 7, -2, 4, -2),   # j=0: out {0,3,6}, in0 [7,5,3], in1 [4,2,0]
    (1, 3, 5, -2, 8, -2),   # j=1: out {1,4,7}, in0 [5,3,1], in1 [8,6,4]
    (2, 3, 0, 1, 6, 1),     # j=2: out {2,5,8}, in0 [0,1,2], in1 [6,7,8]
]

# engine plan: op -> "v" (DVE) / "g" (GPSIMD). GPSIMD's SBUF port is shared
# with the DVE (POOL slot): measured combined V+G throughput during overlap
# equals V-alone, so all 2-input work stays on V; ACT (own port) runs 1-input.
DEFAULT_PLAN = {
    "p0": "v",                               # batched P row 0
    **{f"pp{i}": "v" for i in range(3)},     # P pairs (rows 1,2)
    **{f"q{j}": "v" for j in range(3)},      # batched Q
    "z": "v",
    "tm": "v",
    "det1": "v",
    "det2": "v",
    "s": "v",
    "w": "v",
    "scale": "v",
    **{f"wp{r}": "v" for r in range(3)},
}


def _V(base_ap, off, dims):
    """Strided view of a tile: dims = [(step, count), ...] free dims, last
    dim innermost. Offset in elements."""
    import concourse.bass as bass

    return bass.AP(
        base_ap.tensor,
        base_ap.offset + off,
        [list(base_ap.ap[0])] + [[int(s), int(n)] for s, n in dims],
    )


def build_nc(bl=BL, c=C, plan=None):
    import concourse.bass as bass
    import concourse.bacc as bacc
    import concourse.mybir as mybir
    from concourse.tile import TileContext

    plan = dict(DEFAULT_PLAN, **(plan or {}))
    f32 = mybir.dt.float32
    nch = bl // (P * c)
    assert bl == nch * P * c

    nc = bacc.Bacc()
    # DRAM layout (host-permuted): (nch*128, 12*C) — row = (chunk, partition),
    # 12 planar planes of C floats contiguous per row.
    trf = nc.declare_dram_parameter("trf", [nch * P, 12 * c], f32, isOutput=False)
    out = nc.declare_dram_parameter("out", [nch * P, 12 * c], f32, isOutput=True)
    trf_t = trf.ap().rearrange("(n p) f -> n p f", p=P)
    out_t = out.ap().rearrange("(n p) f -> n p f", p=P)

    eng = None
    state = {}

    def prefetch(n, io):
        # DMA-in + diag (ACT) — issued one stage early so the load and the
        # diag pass are done before the products need them
        t = io.tile([P, 12 * c], f32, tag="t")
        nc.sync.dma_start(out=t[:], in_=trf_t[n])
        d01 = _V(t, 0, [(1, 2 * c)])
        nc.scalar.add(d01, d01, 1.0)
        d5 = _V(t, 5 * c, [(1, c)])
        nc.scalar.add(d5, d5, 1.0)
        state[n] = {"t": t}

    def head(n, io, tmp):
        # all 18 products (V)
        st = state[n]
        t = st["t"]
        po = io.tile([P, 12 * c], f32, tag="po")  # P/Z/O planes 0..8, w 9..11
        qq = tmp.tile([P, 9 * c], f32, tag="qq")  # Q then wp
        st["po"], st["qq"] = po, qq
        eng[plan["p0"]].tensor_mul(
            _V(po, 0, [(c, 3), (1, c)]),
            _V(t, 0, [(4 * c, 3), (1, c)]),
            _V(t, 5 * c, [(c, 3), (1, c)]),
        )
        for i, (ob, os_, a0, s0, b0, s1) in enumerate(P_PAIRS):
            eng[plan[f"pp{i}"]].tensor_mul(
                _V(po, ob * c, [(os_ * c, 2), (1, c)]),
                _V(t, a0 * c, [(s0 * c, 2), (1, c)]),
                _V(t, b0 * c, [(s1 * c, 2), (1, c)]),
            )
        for j, (ob, os_, a0, s0, b0, s1) in enumerate(Q_BATCH):
            eng[plan[f"q{j}"]].tensor_mul(
                _V(qq, ob * c, [(os_ * c, 3), (1, c)]),
                _V(t, a0 * c, [(s0 * c, 3), (1, c)]),
                _V(t, b0 * c, [(s1 * c, 3), (1, c)]),
            )

    def mid(n, tmp):
        # Z, det chain, recip (V); rdet9 fan-out + tneg (ACT)
        st = state[n]
        t, po, qq = st["t"], st["po"], st["qq"]
        pf = _V(po, 0, [(1, 9 * c)])
        eng[plan["z"]].tensor_sub(pf, pf, _V(qq, 0, [(1, 9 * c)]))

        tm = tmp.tile([P, 3 * c], f32, tag="tm")
        st["tm"] = tm
        eng[plan["tm"]].tensor_mul(
            _V(tm, 0, [(c, 3), (1, c)]),
            _V(t, c, [(c, 3), (1, c)]),
            _V(po, 0, [(c, 3), (1, c)]),
        )
        det = tmp.tile([P, c], f32, tag="det")
        eng[plan["det1"]].tensor_add(
            det[:], _V(tm, 0, [(1, c)]), _V(tm, c, [(1, c)])
        )
        eng[plan["det2"]].tensor_add(det[:], det[:], _V(tm, 2 * c, [(1, c)]))

        # rdet = 1/det (single custom DVE op, ~4e-6 rel err; det~1 so no edge
        # cases), replicated to 9 planes via log-doubling ACT copies so the
        # scale stage is one flat 9C op. The copy chain overlaps the next
        # chunk's products on V.
        rdet9 = tmp.tile([P, 9 * c], f32, tag="rdet9")
        st["rdet9"] = rdet9
        nc.vector.reciprocal_approx_fast(_V(rdet9, 0, [(1, c)]), det[:])
        nc.scalar.copy(_V(rdet9, c, [(1, c)]), _V(rdet9, 0, [(1, c)]))
        nc.scalar.copy(_V(rdet9, 2 * c, [(1, 2 * c)]), _V(rdet9, 0, [(1, 2 * c)]))
        nc.scalar.copy(_V(rdet9, 4 * c, [(1, 4 * c)]), _V(rdet9, 0, [(1, 4 * c)]))
        nc.scalar.copy(_V(rdet9, 8 * c, [(1, c)]), _V(rdet9, 0, [(1, c)]))

        # tneg: t planes 9..11 *= -1 (in place, ACT)
        tp = _V(t, 9 * c, [(1, 3 * c)])
        nc.scalar.mul(tp, tp, -1.0)

    def tail(n):
        # scale, wp, w sums, output DMAs
        st = state.pop(n)
        t, po, qq, tm, rdet9 = st["t"], st["po"], st["qq"], st["tm"], st["rdet9"]
        eng[plan["scale"]].tensor_mul(
            _V(po, 0, [(1, 9 * c)]),
            _V(po, 0, [(1, 9 * c)]),
            _V(rdet9, 0, [(1, 9 * c)]),
        )
        for r in range(3):
            eng[plan[f"wp{r}"]].tensor_mul(
                _V(qq, 3 * r * c, [(1, 3 * c)]),
                _V(po, 3 * r * c, [(1, 3 * c)]),
                _V(t, 9 * c, [(1, 3 * c)]),
            )
        # O block can ship while the w tail computes
        nc.sync.dma_start(
            out=_V(out_t[n], 0, [(1, 9 * c)]), in_=_V(po, 0, [(1, 9 * c)])
        )
        # w_r = wp[3r] + wp[3r+1] + wp[3r+2] -> po planes 9..11
        # (s scratch reuses tm, dead after the det sums)
        eng[plan["s"]].tensor_add(
            _V(tm, 0, [(c, 3), (1, c)]),
            _V(qq, 0, [(3 * c, 3), (1, c)]),
            _V(qq, c, [(3 * c, 3), (1, c)]),
        )
        eng[plan["w"]].tensor_add(
            _V(po, 9 * c, [(c, 3), (1, c)]),
            _V(tm, 0, [(c, 3), (1, c)]),
            _V(qq, 2 * c, [(3 * c, 3), (1, c)]),
        )
        nc.sync.dma_start(
            out=_V(out_t[n], 9 * c, [(1, 3 * c)]),
            in_=_V(po, 9 * c, [(1, 3 * c)]),
        )

    with TileContext(nc) as tc:
        with (
            tc.tile_pool(name="io", bufs=2) as io,
            tc.tile_pool(name="tmp", bufs=2) as tmp,
        ):
            eng = {"v": nc.vector, "g": nc.gpsimd}
            # software pipeline: V runs chunk n+1's products while ACT does
            # chunk n's rdet9 fan-out, so V never stalls on the copy chain.
            # prefetch(n+1) is emitted after tail(n-1) so every prior-chunk
            # consumer of its recycled buffers is already known to Tile.
            prefetch(0, io)
            for n in range(nch):
                head(n, io, tmp)
                if n >= 1:
                    tail(n - 1)
                mid(n, tmp)
                if n + 1 < nch:
                    prefetch(n + 1, io)
            tail(nch - 1)

    return nc


_CACHE = {}


def _get_nc():
    if "nc" not in _CACHE:
        nc = build_nc()
        nc.finalize()
        _CACHE["nc"] = nc
    return _CACHE["nc"]


def _shard_inputs(trf):
    """(B,3,4) -> per-core (nch*128, 12*C) planar slabs."""
    x = np.ascontiguousarray(np.asarray(trf, dtype=np.float32)).reshape(
        NCORES, NCH, P, C, 12
    )
    # permute matrix positions into plane slots, planes outer, matrices inner
    xp = x[:, :, :, :, POS].transpose(0, 1, 2, 4, 3)  # (8, nch, 128, 12, C)
    xp = np.ascontiguousarray(xp).reshape(NCORES, NCH * P, 12 * C)
    return xp


def _unshard_output(outs):
    """per-core (nch*128, 12*C) planar -> (B, 3, 4)."""
    o = outs.reshape(NCORES, NCH, P, 12, C).transpose(0, 1, 2, 4, 3)
    full = np.empty((NCORES, NCH, P, C, 12), dtype=np.float32)
    full[..., OPOS] = o
    return full.reshape(B, 3, 4)


def run(trf, trace=False, **spmd_kwargs):
    """Shard, run on 8 cores, gather. Returns (output, BassKernelResults)."""
    from concourse.bass_utils import run_bass_kernel_spmd

    xp = _shard_inputs(trf)
    in_maps = [{"trf": xp[i]} for i in range(NCORES)]
    nc = _get_nc()
    res = run_bass_kernel_spmd(
        nc, in_maps, list(range(NCORES)), trace=trace, **spmd_kwargs
    )
    outs = np.stack([np.asarray(res.results[i]["out"]) for i in range(NCORES)])
    return _unshard_output(outs).astype(np.float32), res


def kernel(trf):
    return run(trf)[0]


# revision 14
# speedup vs baseline: 1.0264x; 1.0264x over previous
"""Trainium2 Bass kernel: batched inverse of homogeneous affine transforms.

Problem: trf (B, 3, 4) fp32 "shift" affines. Padded M = [[I3 + dA, t], [0, 1]].
Output = top 3 rows of M^-1 = [A^-1 | -A^-1 t] where A = I3 + dA.

Closed form via the column-cross-product adjugate:
    Z[3r+j]  = P[3r+j] - Q[3r+j]   (cross(a_{r+1}, a_{r+2}) components)
    det      = a1 . Z[0:3] ; O = Z * (1/det) ; w_r = sum_j O[r][j] * (-t_j)

Layout: PLANAR per partition. Host pre-permutes each core's (BL, 12) slab to
(nch, 128, 12, C): partition p holds 12 contiguous planes of C consecutive
matrices. Every engine op then runs on dense step-1 inner runs (measured ~2x
faster on DVE than the stride-12 interleaved layout) while each DMA still
moves one contiguous 24KB run per partition.

Plane permutation POS (slot -> matrix position) was chosen by combinatorial
search so the 9 Q-products batch as 3 ops, P row 0 batches, and tm batches
(arithmetic-progression plane strides). Work is split DVE / GPSIMD / ACT to
balance engine busy time; all pools are double-buffered so chunks pipeline.
"""

import numpy as np

B = 4_194_304
NCORES = 8
BL = B // NCORES  # 524288 matrices per core
P = 128
C = 512           # matrices per partition per chunk
NCH = BL // (P * C)  # 8 chunks

# slot -> input position (position = 4*r + col, row-major (3,4))
POS = [5, 0, 4, 8, 9, 10, 2, 6, 1, 3, 7, 11]
# output plane k -> output position: planes 0..8 = O[r][j] at 4r+j, 9..11 = w_r
OPOS = [0, 1, 2, 4, 5, 6, 8, 9, 10, 3, 7, 11]

# P products (out plane 3r+j in po block), as (out, in0_slot, in1_slot):
# row 0 batched: out {0,1,2}, in0 [0,4,8] (step 4), in1 [5,6,7] (step 1)
# rows 1,2 as pairs (2-term progressions are always affine):
# (out_base, out_step, in0_base, in0_step, in1_base, in1_step)
P_PAIRS = [
    (3, 1, 7, -2, 3, -2),   # (3,7,3), (4,5,1)
    (5, 1, 6, -4, 2, 2),    # (5,6,2), (6,2,4)
    (7, 1, 3, -2, 8, -8),   # (7,3,8), (8,1,0)
]
# Q products batched by j: (out_base, out_step, in0_base, in0_step, in1_base, in1_step)
Q_BATCH = [
    (0, 3, 7, -2, 4, -2),   # j=0: out {0,3,6}, in0 [7,5,3], in1 [4,2,0]
    (1, 3, 5, -2, 8, -2),   # j=1: out {1,4,7}, in0 [5,3,1], in1 [8,6,4]
    (2, 3, 0, 1, 6, 1),     # j=2: out {2,5,8}, in0 [0,1,2], in1 [6,7,8]
]

# engine plan: op -> "v" (DVE) / "g" (GPSIMD). GPSIMD's SBUF port is shared
# with the DVE (POOL slot): measured combined V+G throughput during overlap
# equals V-alone, so all 2-input work stays on V; ACT (own port) runs 1-input.
DEFAULT_PLAN = {
    "p0": "v",                               # batched P row 0
    **{f"pp{i}": "v" for i in range(3)},     # P pairs (rows 1,2)
    **{f"q{j}": "v" for j in range(3)},      # batched Q
    "z": "v",
    "tm": "v",
    "det1": "v",
    "det2": "v",
    "s": "v",
    "w": "v",
    "scale": "v",
    **{f"wp{r}": "v" for r in range(3)},
}


def _V(base_ap, off, dims):
    """Strided view of a tile: dims = [(step, count), ...] free dims, last
    dim innermost. Offset in elements."""
    import concourse.bass as bass

    return bass.AP(
        base_ap.tensor,
        base_ap.offset + off,
        [list(base_ap.ap[0])] + [[int(s), int(n)] for s, n in dims],
    )


def build_nc(bl=BL, c=C, plan=None):
    import concourse.bass as bass
    import concourse.bacc as bacc
    import concourse.mybir as mybir
    from concourse.tile import TileContext

    plan = dict(DEFAULT_PLAN, **(plan or {}))
    f32 = mybir.dt.float32
    nch = bl // (P * c)
    assert bl == nch * P * c

    nc = bacc.Bacc()
    # DRAM layout (host-permuted): (nch*128, 12*C) — row = (chunk, partition),
    # 12 planar planes of C floats contiguous per row.
    trf = nc.declare_dram_parameter("trf", [nch * P, 12 * c], f32, isOutput=False)
    out = nc.declare_dram_parameter("out", [nch * P, 12 * c], f32, isOutput=True)
    trf_t = trf.ap().rearrange("(n p) f -> n p f", p=P)
    out_t = out.ap().rearrange("(n p) f -> n p f", p=P)

    eng = None
    state = {}

    def prefetch(n, tio):
        # DMA-in + diag (ACT) — issued one stage early so the load and the
        # diag pass are done before the products need them
        t = tio.tile([P, 12 * c], f32, tag="t")
        nc.sync.dma_start(out=t[:], in_=trf_t[n])
        d01 = _V(t, 0, [(1, 2 * c)])
        nc.scalar.add(d01, d01, 1.0)
        d5 = _V(t, 5 * c, [(1, c)])
        nc.scalar.add(d5, d5, 1.0)
        state[n] = {"t": t}

    def head(n, io, tmp):
        # all 18 products (V)
        st = state[n]
        t = st["t"]
        po = io.tile([P, 12 * c], f32, tag="po")  # P/Z/O planes 0..8, w 9..11
        qq = tmp.tile([P, 9 * c], f32, tag="qq")  # Q then wp
        st["po"], st["qq"] = po, qq
        eng[plan["p0"]].tensor_mul(
            _V(po, 0, [(c, 3), (1, c)]),
            _V(t, 0, [(4 * c, 3), (1, c)]),
            _V(t, 5 * c, [(c, 3), (1, c)]),
        )
        for i, (ob, os_, a0, s0, b0, s1) in enumerate(P_PAIRS):
            eng[plan[f"pp{i}"]].tensor_mul(
                _V(po, ob * c, [(os_ * c, 2), (1, c)]),
                _V(t, a0 * c, [(s0 * c, 2), (1, c)]),
                _V(t, b0 * c, [(s1 * c, 2), (1, c)]),
            )
        for j, (ob, os_, a0, s0, b0, s1) in enumerate(Q_BATCH):
            eng[plan[f"q{j}"]].tensor_mul(
                _V(qq, ob * c, [(os_ * c, 3), (1, c)]),
                _V(t, a0 * c, [(s0 * c, 3), (1, c)]),
                _V(t, b0 * c, [(s1 * c, 3), (1, c)]),
            )

    def mid(n, tmp):
        # Z, det chain, recip (V); rdet9 fan-out + tneg (ACT)
        st = state[n]
        t, po, qq = st["t"], st["po"], st["qq"]
        pf = _V(po, 0, [(1, 9 * c)])
        eng[plan["z"]].tensor_sub(pf, pf, _V(qq, 0, [(1, 9 * c)]))

        tm = tmp.tile([P, 3 * c], f32, tag="tm")
        st["tm"] = tm
        eng[plan["tm"]].tensor_mul(
            _V(tm, 0, [(c, 3), (1, c)]),
            _V(t, c, [(c, 3), (1, c)]),
            _V(po, 0, [(c, 3), (1, c)]),
        )
        det = tmp.tile([P, c], f32, tag="det")
        eng[plan["det1"]].tensor_add(
            det[:], _V(tm, 0, [(1, c)]), _V(tm, c, [(1, c)])
        )
        eng[plan["det2"]].tensor_add(det[:], det[:], _V(tm, 2 * c, [(1, c)]))

        # rdet = 1/det (single custom DVE op, ~4e-6 rel err; det~1 so no edge
        # cases), replicated to 3 planes by two single-hop copies on two
        # different engines (ACT + GPSIMD) so the replication latency between
        # recip and scale is one copy, not a serial chain.
        rdet3 = tmp.tile([P, 3 * c], f32, tag="rdet3")
        st["rdet3"] = rdet3
        nc.vector.reciprocal_approx_fast(_V(rdet3, 0, [(1, c)]), det[:])
        nc.scalar.copy(_V(rdet3, c, [(1, c)]), _V(rdet3, 0, [(1, c)]))
        nc.gpsimd.tensor_copy(_V(rdet3, 2 * c, [(1, c)]), _V(rdet3, 0, [(1, c)]))

        # tneg: t planes 9..11 *= -1 (in place, ACT)
        tp = _V(t, 9 * c, [(1, 3 * c)])
        nc.scalar.mul(tp, tp, -1.0)

    def tail(n):
        # scale, wp, w sums, output DMAs
        st = state.pop(n)
        t, po, qq, tm, rdet3 = st["t"], st["po"], st["qq"], st["tm"], st["rdet3"]
        for r in range(3):
            eng[plan["scale"]].tensor_mul(
                _V(po, 3 * r * c, [(1, 3 * c)]),
                _V(po, 3 * r * c, [(1, 3 * c)]),
                _V(rdet3, 0, [(1, 3 * c)]),
            )
        for r in range(3):
            eng[plan[f"wp{r}"]].tensor_mul(
                _V(qq, 3 * r * c, [(1, 3 * c)]),
                _V(po, 3 * r * c, [(1, 3 * c)]),
                _V(t, 9 * c, [(1, 3 * c)]),
            )
        # O block can ship while the w tail computes
        nc.sync.dma_start(
            out=_V(out_t[n], 0, [(1, 9 * c)]), in_=_V(po, 0, [(1, 9 * c)])
        )
        # w_r = wp[3r] + wp[3r+1] + wp[3r+2] -> po planes 9..11
        # (s scratch reuses tm, dead after the det sums)
        eng[plan["s"]].tensor_add(
            _V(tm, 0, [(c, 3), (1, c)]),
            _V(qq, 0, [(3 * c, 3), (1, c)]),
            _V(qq, c, [(3 * c, 3), (1, c)]),
        )
        eng[plan["w"]].tensor_add(
            _V(po, 9 * c, [(c, 3), (1, c)]),
            _V(tm, 0, [(c, 3), (1, c)]),
            _V(qq, 2 * c, [(3 * c, 3), (1, c)]),
        )
        nc.sync.dma_start(
            out=_V(out_t[n], 9 * c, [(1, 3 * c)]),
            in_=_V(po, 9 * c, [(1, 3 * c)]),
        )

    with TileContext(nc) as tc:
        with (
            tc.tile_pool(name="tio", bufs=3) as tio,
            tc.tile_pool(name="io", bufs=2) as io,
            tc.tile_pool(name="tmp", bufs=2) as tmp,
        ):
            eng = {"v": nc.vector, "g": nc.gpsimd}
            # software pipeline: V runs chunk n+1's products while ACT does
            # chunk n's rdet9 fan-out, so V never stalls on the copy chain.
            # prefetch(n+1) is emitted after tail(n-1) so every prior-chunk
            # consumer of its recycled buffers is already known to Tile.
            prefetch(0, tio)
            for n in range(nch):
                head(n, io, tmp)
                if n >= 1:
                    tail(n - 1)
                mid(n, tmp)
                if n + 1 < nch:
                    prefetch(n + 1, tio)
            tail(nch - 1)

    return nc


_CACHE = {}


def _get_nc():
    if "nc" not in _CACHE:
        nc = build_nc()
        nc.finalize()
        _CACHE["nc"] = nc
    return _CACHE["nc"]


def _shard_inputs(trf):
    """(B,3,4) -> per-core (nch*128, 12*C) planar slabs."""
    x = np.ascontiguousarray(np.asarray(trf, dtype=np.float32)).reshape(
        NCORES, NCH, P, C, 12
    )
    # permute matrix positions into plane slots, planes outer, matrices inner
    xp = x[:, :, :, :, POS].transpose(0, 1, 2, 4, 3)  # (8, nch, 128, 12, C)
    xp = np.ascontiguousarray(xp).reshape(NCORES, NCH * P, 12 * C)
    return xp


def _unshard_output(outs):
    """per-core (nch*128, 12*C) planar -> (B, 3, 4)."""
    o = outs.reshape(NCORES, NCH, P, 12, C).transpose(0, 1, 2, 4, 3)
    full = np.empty((NCORES, NCH, P, C, 12), dtype=np.float32)
    full[..., OPOS] = o
    return full.reshape(B, 3, 4)


def run(trf, trace=False, **spmd_kwargs):
    """Shard, run on 8 cores, gather. Returns (output, BassKernelResults)."""
    from concourse.bass_utils import run_bass_kernel_spmd

    xp = _shard_inputs(trf)
    in_maps = [{"trf": xp[i]} for i in range(NCORES)]
    nc = _get_nc()
    res = run_bass_kernel_spmd(
        nc, in_maps, list(range(NCORES)), trace=trace, **spmd_kwargs
    )
    outs = np.stack([np.asarray(res.results[i]["out"]) for i in range(NCORES)])
    return _unshard_output(outs).astype(np.float32), res


def kernel(trf):
    return run(trf)[0]
